# revision 9
# baseline (speedup 1.0000x reference)
"""Trainium2 Bass/Tile kernel for the HairBundle SDE drift+diffusion.

Contract: kernel(t, x) takes the FULL inputs (t: [1] f32, x: [8_000_000, 5]
f32) and returns the full (drift, diffusion) pair, matching reference().

Strategy
--------
Trivially data-parallel over the sample-path axis: 8 NeuronCores, each core
takes 1M rows, padded to 128*7813 rows and laid out [128 partitions, 7813
rows/partition, 5 components] so every DMA is dense (contiguous 5*F floats
per partition).  Per 128xFx5 tile the drift is evaluated with 9 VectorE
streams (tensor_tensor / scalar_tensor_tensor) + 9 ScalarE(ACT) streams
(sigmoid + affine Identity ops), reading/writing the interleaved component
planes through stride-5 access patterns.  The diffusion output is a
constant broadcast and is produced host-side for free.

Math (constants folded from the reference):
    d  = h - a;  po = sigmoid(4 d)
    dh = 0.375*(2*(-1.8 h + a) + po) + force         (ACT bias = force)
    da = -0.06*((5/3) dh + (h + 0.75 a - 0.525 m)) + (0.1*force - 0.035)
    dv = (v - 1)*(-c*po - k) - k   for (v,c,k) in
         (m,1.2,0.8), (g,0.7,0.5), (t,0.3,0.4)
    force = 0.5*sin(2*pi*t)
"""

import numpy as np

_B = 8_000_000
_NCORES = 8
_RPC = _B // _NCORES            # rows per core = 1_000_000
_P = 128
_Q = -(-_RPC // _P)             # 7813 rows per partition (padded)
_PADROWS = _P * _Q - _RPC       # 64 junk rows at the tail of each shard
_F = 768                        # rows-per-partition per SBUF tile
_DSIG = np.array([0.05, 0.02, 0.0, 0.0, 0.0], dtype=np.float32)

_CACHE = {}


def _build_nc(q, f):
    """Build the per-core Bass program for [128, q, 5] in/out, tile width f."""
    import concourse.bacc as bacc
    import concourse.mybir as mybir
    import concourse.tile as tile

    f32 = mybir.dt.float32
    Act = mybir.ActivationFunctionType
    Op = mybir.AluOpType

    nc = bacc.Bacc("TRN2", debug=False)
    x_d = nc.dram_tensor("x", [_P, q, 5], f32, kind="ExternalInput").ap()
    c_d = nc.dram_tensor("consts", [_P, 5], f32, kind="ExternalInput").ap()
    o_d = nc.dram_tensor("drift", [_P, q, 5], f32, kind="ExternalOutput").ap()

    ntiles = -(-q // f)

    with tile.TileContext(nc) as tc:
        with (
            tc.tile_pool(name="io", bufs=3) as io_pool,
            tc.tile_pool(name="tmp", bufs=2) as tmp_pool,
            tc.tile_pool(name="cst", bufs=1) as cst_pool,
        ):
            consts = cst_pool.tile([_P, 5], f32, name="consts_sb")
            nc.sync.dma_start(consts[:, :], c_d[:, :])
            force_b = consts[:, 0:1]
            cprime_b = consts[:, 1:2]
            km_b = consts[:, 2:3]   # -0.8
            kg_b = consts[:, 3:4]   # -0.5
            kt_b = consts[:, 4:5]   # -0.4

            for ti in range(ntiles):
                f0 = ti * f
                fw = min(f, q - f0)

                X = io_pool.tile([_P, f, 5], f32, tag="X", name="X")
                nc.sync.dma_start(X[:, :fw, :], x_d[:, f0 : f0 + fw, :])
                D = io_pool.tile([_P, f, 5], f32, tag="D", name="D")

                h = X[:, :fw, 0]
                a = X[:, :fw, 1]
                m = X[:, :fw, 2]
                g = X[:, :fw, 3]
                t_ = X[:, :fw, 4]
                dh = D[:, :fw, 0]
                da = D[:, :fw, 1]
                dm = D[:, :fw, 2]
                dg = D[:, :fw, 3]
                dt = D[:, :fw, 4]

                def T(nm):
                    return tmp_pool.tile([_P, f], f32, tag=nm, name=nm)[:, :fw]

                d = T("d")
                po = T("po")
                u1 = T("u1")
                u2 = T("u2")
                z1 = T("z1")
                z2 = T("z2")
                zp = T("zp")
                qm = T("qm")
                qg = T("qg")
                qt = T("qt")
                pm = T("pm")
                pg = T("pg")
                pt = T("pt")

                # d = h - a ; po = sigmoid(4 d)
                nc.vector.tensor_tensor(d, h, a, Op.subtract)
                nc.scalar.activation(po, d, Act.Sigmoid, scale=4.0)

                # dh = 0.375*(2*(a - 1.8 h) + po) + force
                nc.vector.scalar_tensor_tensor(u1, h, -1.8, a, Op.mult, Op.add)
                nc.vector.scalar_tensor_tensor(u2, u1, 2.0, po, Op.mult, Op.add)
                nc.scalar.activation(dh, u2, Act.Identity, bias=force_b, scale=0.375)

                # da = -0.06*((5/3) dh + h + 0.75 a - 0.525 m) + (0.1 force - 0.035)
                nc.vector.scalar_tensor_tensor(z1, a, 0.75, h, Op.mult, Op.add)
                nc.vector.scalar_tensor_tensor(z2, m, -0.525, z1, Op.mult, Op.add)
                nc.vector.scalar_tensor_tensor(zp, dh, 5.0 / 3.0, z2, Op.mult, Op.add)
                nc.scalar.activation(da, zp, Act.Identity, bias=cprime_b, scale=-0.06)

                # dv = (v-1)*(-c po - k) - k
                nc.scalar.activation(qm, po, Act.Identity, bias=km_b, scale=-1.2)
                nc.scalar.activation(qg, po, Act.Identity, bias=kg_b, scale=-0.7)
                nc.scalar.activation(qt, po, Act.Identity, bias=kt_b, scale=-0.3)
                nc.vector.scalar_tensor_tensor(pm, m, 1.0, qm, Op.subtract, Op.mult)
                nc.vector.scalar_tensor_tensor(pg, g, 1.0, qg, Op.subtract, Op.mult)
                nc.vector.scalar_tensor_tensor(pt, t_, 1.0, qt, Op.subtract, Op.mult)
                nc.scalar.activation(dm, pm, Act.Identity, bias=km_b, scale=1.0)
                nc.scalar.activation(dg, pg, Act.Identity, bias=kg_b, scale=1.0)
                nc.scalar.activation(dt, pt, Act.Identity, bias=kt_b, scale=1.0)

                nc.sync.dma_start(o_d[:, f0 : f0 + fw, :], D[:, :fw, :])

    nc.compile()
    return nc


def _get_nc():
    key = (_Q, _F)
    if key not in _CACHE:
        _CACHE[key] = _build_nc(_Q, _F)
    return _CACHE[key]


def _run_device(x, force, trace=False, tmpdir=None):
    """Shard x [8M,5] over 8 cores, run the Bass kernel, gather drift."""
    from concourse.bass_utils import run_bass_kernel_spmd

    nc = _get_nc()

    consts_np = np.empty((_P, 5), dtype=np.float32)
    consts_np[:, 0] = force
    consts_np[:, 1] = 0.1 * force - 0.035
    consts_np[:, 2] = -0.8
    consts_np[:, 3] = -0.5
    consts_np[:, 4] = -0.4

    in_maps = []
    for i in range(_NCORES):
        shard = np.zeros((_P * _Q, 5), dtype=np.float32)
        shard[:_RPC] = x[i * _RPC : (i + 1) * _RPC]
        in_maps.append({"x": shard.reshape(_P, _Q, 5), "consts": consts_np})

    res = run_bass_kernel_spmd(
        nc, in_maps, list(range(_NCORES)), trace=trace, tmpdir=tmpdir
    )

    drift = np.empty((_B, 5), dtype=np.float32)
    for i in range(_NCORES):
        drift[i * _RPC : (i + 1) * _RPC] = res.results[i]["drift"].reshape(
            _P * _Q, 5
        )[:_RPC]
    return drift, res


def kernel(t, x):
    t = np.asarray(t, dtype=np.float32)
    x = np.ascontiguousarray(np.asarray(x, dtype=np.float32))
    force = np.float32(0.5 * np.sin(6.283185307179586 * float(t[0]) + 0.0))
    drift, _ = _run_device(x, force, trace=False)
    diffusion = np.broadcast_to(_DSIG, x.shape)
    return drift, diffusion


# revision 12
# speedup vs baseline: 1.2453x; 1.2453x over previous
"""Trainium2 Bass/Tile kernel for the HairBundle SDE drift+diffusion.

Contract: kernel(t, x) takes the FULL inputs (t: [1] f32, x: [8_000_000, 5]
f32) and returns the full (drift, diffusion) pair, matching reference().

Strategy
--------
Trivially data-parallel over the sample-path axis: 8 NeuronCores, each core
takes 1M rows, padded to 128*7813 rows and laid out [128 partitions, 7813
rows/partition, 5 components] so every DMA is dense (contiguous 5*F floats
per partition).  Per 128xFx5 tile the drift is evaluated with 9 VectorE
streams (tensor_tensor / scalar_tensor_tensor) + 9 ScalarE(ACT) streams
(sigmoid + affine Identity ops), reading/writing the interleaved component
planes through stride-5 access patterns.  The diffusion output is a
constant broadcast and is produced host-side for free.

Math (constants folded from the reference):
    d  = h - a;  po = sigmoid(4 d)
    dh = 0.375*(2*(-1.8 h + a) + po) + force         (ACT bias = force)
    da = -0.06*((5/3) dh + (h + 0.75 a - 0.525 m)) + (0.1*force - 0.035)
    dv = (v - 1)*(-c*po - k) - k   for (v,c,k) in
         (m,1.2,0.8), (g,0.7,0.5), (t,0.3,0.4)
    force = 0.5*sin(2*pi*t)
"""

import numpy as np

_B = 8_000_000
_NCORES = 8
_RPC = _B // _NCORES            # rows per core = 1_000_000
_P = 128
_Q = -(-_RPC // _P)             # 7813 rows per partition (padded)
_PADROWS = _P * _Q - _RPC       # 64 junk rows at the tail of each shard
_F = 768                        # rows-per-partition per SBUF tile
_DSIG = np.array([0.05, 0.02, 0.0, 0.0, 0.0], dtype=np.float32)

_CACHE = {}


def _build_nc(q, f):
    """Build the per-core Bass program for [128, q, 5] in/out, tile width f."""
    import concourse.bacc as bacc
    import concourse.mybir as mybir
    import concourse.tile as tile

    f32 = mybir.dt.float32
    Act = mybir.ActivationFunctionType
    Op = mybir.AluOpType

    nc = bacc.Bacc("TRN2", debug=False)
    x_d = nc.dram_tensor("x", [_P, q, 5], f32, kind="ExternalInput").ap()
    c_d = nc.dram_tensor("consts", [_P, 5], f32, kind="ExternalInput").ap()
    o_d = nc.dram_tensor("drift", [_P, q, 5], f32, kind="ExternalOutput").ap()

    ntiles = -(-q // f)

    with tile.TileContext(nc) as tc:
        with (
            tc.tile_pool(name="io", bufs=3) as io_pool,
            tc.tile_pool(name="tmp", bufs=2) as tmp_pool,
            tc.tile_pool(name="cst", bufs=1) as cst_pool,
        ):
            consts = cst_pool.tile([_P, 5], f32, name="consts_sb")
            nc.sync.dma_start(consts[:, :], c_d[:, :])
            force_b = consts[:, 0:1]
            cprime_b = consts[:, 1:2]
            km_b = consts[:, 2:3]   # -0.8
            kg_b = consts[:, 3:4]   # -0.5
            kt_b = consts[:, 4:5]   # -0.4

            for ti in range(ntiles):
                f0 = ti * f
                fw = min(f, q - f0)

                X = io_pool.tile([_P, f, 5], f32, tag="X", name="X")
                nc.sync.dma_start(X[:, :fw, :], x_d[:, f0 : f0 + fw, :])
                D = io_pool.tile([_P, f, 5], f32, tag="D", name="D")

                h = X[:, :fw, 0]
                a = X[:, :fw, 1]
                m = X[:, :fw, 2]
                g = X[:, :fw, 3]
                t_ = X[:, :fw, 4]
                dh = D[:, :fw, 0]
                da = D[:, :fw, 1]
                dm = D[:, :fw, 2]
                dg = D[:, :fw, 3]
                dt = D[:, :fw, 4]

                def T(nm):
                    return tmp_pool.tile([_P, f], f32, tag=nm, name=nm)[:, :fw]

                d = T("d")
                po = T("po")
                u1 = T("u1")
                u2 = T("u2")
                z1 = T("z1")
                z2 = T("z2")
                zp = T("zp")
                qm = T("qm")
                qg = T("qg")
                qt = T("qt")
                pm = T("pm")
                pg = T("pg")
                pt = T("pt")

                # d = h - a ; po = sigmoid(4 d)
                nc.vector.tensor_tensor(d, h, a, Op.subtract)
                nc.scalar.activation(po, d, Act.Sigmoid, scale=4.0)

                # dh = 0.375*(2*(a - 1.8 h) + po) + force
                nc.vector.scalar_tensor_tensor(u1, h, -1.8, a, Op.mult, Op.add)
                nc.vector.scalar_tensor_tensor(u2, u1, 2.0, po, Op.mult, Op.add)
                nc.scalar.activation(dh, u2, Act.Identity, bias=force_b, scale=0.375)

                # da = 0.0375*(2h - 3.2a - po + 0.84m) - 0.035
                nc.vector.scalar_tensor_tensor(z1, a, -1.6, h, Op.mult, Op.add)
                nc.vector.scalar_tensor_tensor(z2, z1, 2.0, po, Op.mult, Op.subtract)
                nc.vector.scalar_tensor_tensor(zp, m, 0.84, z2, Op.mult, Op.add)
                nc.scalar.activation(da, zp, Act.Identity, bias=cprime_b, scale=0.0375)

                # dv = (v-1)*(-c po - k) - k
                nc.scalar.activation(qm, po, Act.Identity, bias=km_b, scale=-1.2)
                nc.scalar.activation(qg, po, Act.Identity, bias=kg_b, scale=-0.7)
                nc.scalar.activation(qt, po, Act.Identity, bias=kt_b, scale=-0.3)
                nc.vector.scalar_tensor_tensor(pm, m, 1.0, qm, Op.subtract, Op.mult)
                nc.vector.scalar_tensor_tensor(pg, g, 1.0, qg, Op.subtract, Op.mult)
                nc.vector.scalar_tensor_tensor(pt, t_, 1.0, qt, Op.subtract, Op.mult)
                nc.scalar.activation(dm, pm, Act.Identity, bias=km_b, scale=1.0)
                nc.scalar.activation(dg, pg, Act.Identity, bias=kg_b, scale=1.0)
                nc.scalar.activation(dt, pt, Act.Identity, bias=kt_b, scale=1.0)

                # out-DMA on the (otherwise idle) gpsimd SWDGE queue so the
                # wait-on-compute doesn't block the sync queue's in-DMAs
                nc.gpsimd.dma_start(o_d[:, f0 : f0 + fw, :], D[:, :fw, :])

    nc.compile()
    return nc


def _get_nc():
    key = (_Q, _F)
    if key not in _CACHE:
        _CACHE[key] = _build_nc(_Q, _F)
    return _CACHE[key]


def _run_device(x, force, trace=False, tmpdir=None):
    """Shard x [8M,5] over 8 cores, run the Bass kernel, gather drift."""
    from concourse.bass_utils import run_bass_kernel_spmd

    nc = _get_nc()

    consts_np = np.empty((_P, 5), dtype=np.float32)
    consts_np[:, 0] = force
    consts_np[:, 1] = -0.035
    consts_np[:, 2] = -0.8
    consts_np[:, 3] = -0.5
    consts_np[:, 4] = -0.4

    in_maps = []
    for i in range(_NCORES):
        shard = np.zeros((_P * _Q, 5), dtype=np.float32)
        shard[:_RPC] = x[i * _RPC : (i + 1) * _RPC]
        in_maps.append({"x": shard.reshape(_P, _Q, 5), "consts": consts_np})

    res = run_bass_kernel_spmd(
        nc, in_maps, list(range(_NCORES)), trace=trace, tmpdir=tmpdir
    )

    drift = np.empty((_B, 5), dtype=np.float32)
    for i in range(_NCORES):
        drift[i * _RPC : (i + 1) * _RPC] = res.results[i]["drift"].reshape(
            _P * _Q, 5
        )[:_RPC]
    return drift, res


def kernel(t, x):
    t = np.asarray(t, dtype=np.float32)
    x = np.ascontiguousarray(np.asarray(x, dtype=np.float32))
    force = np.float32(0.5 * np.sin(6.283185307179586 * float(t[0]) + 0.0))
    drift, _ = _run_device(x, force, trace=False)
    diffusion = np.broadcast_to(_DSIG, x.shape)
    return drift, diffusion
